# revision 1
# baseline (speedup 1.0000x reference)
"""Trainium2 Bass kernel for LMPNN-style GNN message passing + entity double-matmul.

Reference computation:
    msg      = (x[src] + rel_emb[rel]) * (1 - 2*neg)        # [E, D]
    aggr_out = segment_sum(msg, dst, N)                     # [N, D]
    aggr     = 0.1*x + aggr_out
    score    = relu((aggr @ E^T) * scale + bias)            # [N, V]
    out      = score @ E                                    # [N, D]

Strategy (8 NeuronCores, node-sharded, no collectives):
  * Core c owns nodes [c*512, (c+1)*512).
  * Message passing is re-expressed densely:  aggr = A @ x + R @ rel_emb,
    where A[n, m] = sum of (1-2*neg) over edges m->n  (+0.1 on the diagonal
    for the residual term) and R[n, r] = sum of (1-2*neg) over edges with
    relation r landing on n. The host builds the integer-valued A/R count
    matrices from the index tensors (pure index preprocessing); the device
    does all floating-point work as dense TensorEngine matmuls accumulated
    in fp32 PSUM, producing aggrT [D, 512] directly.
  * The double matmul streams the (host-transposed / host-swizzled) entity
    table from HBM in bf16, interleaving per-128-entity chunks:
    scoreT = ET_chunk(lhsT) x aggrT -> relu(+scale/bias) on ACT/DVE ->
    outT += E_chunk(lhsT) x scoreT accumulated in a single PSUM bank.
  * Output is outT [128, 512] fp32 per core; host transposes/concats.
"""

import sys

import numpy as np

try:
    import concourse.bass as bass
except ImportError:  # pragma: no cover
    sys.path.insert(0, "/opt/trn_rl_repo")
    import concourse.bass as bass

import ml_dtypes

import concourse.bacc as bacc
import concourse.mybir as mybir
import concourse.tile as tile
from concourse.bass_utils import run_bass_kernel_spmd

BF16 = ml_dtypes.bfloat16
F32 = np.float32


class Cfg:
    def __init__(self, N=4096, E=262144, D=128, R=1000, V=50000, C=8):
        self.N, self.E, self.D, self.R, self.V, self.C = N, E, D, R, V, C
        self.NPC = N // C                       # nodes per core
        assert self.NPC % 128 == 0 and N % 128 == 0
        self.RPAD = ((R + 127) // 128) * 128    # padded relation count
        self.VPAD = ((V + 511) // 512) * 512    # padded entity count
        self.NV = self.VPAD // 128              # 128-entity chunks
        self.NKX = N // 128                     # k-chunks for A @ x
        self.NKR = self.RPAD // 128             # k-chunks for R @ rel


def host_prep(cfg, x, edge_index, relation_id, neg_flag, rel_emb, entity_emb,
              scale, bias):
    """Build per-core in_maps. The host only converts the edge/index tensors
    into dense count matrices + does layout/dtype conversion; all FP math on
    the embeddings happens on device."""
    src = np.asarray(edge_index[0]).astype(np.int64)
    dst = np.asarray(edge_index[1]).astype(np.int64)
    rel = np.asarray(relation_id).astype(np.int64)
    neg = np.asarray(neg_flag).astype(np.int64)
    x = np.asarray(x, F32)
    rel_emb = np.asarray(rel_emb, F32)
    entity_emb = np.asarray(entity_emb, F32)
    scale = np.asarray(scale, F32)
    bias = np.asarray(bias, F32)

    C, NPC, D = cfg.C, cfg.NPC, cfg.D
    negc = (1.0 - 2.0 * neg).astype(F32)

    # dense message-passing operators (index preprocessing)
    A = np.zeros((cfg.N, cfg.N), F32)
    np.add.at(A, (dst, src), negc)
    A[np.arange(cfg.N), np.arange(cfg.N)] += 0.1          # residual 0.1*x
    Rm = np.zeros((cfg.N, cfg.RPAD), F32)
    np.add.at(Rm, (dst, rel), negc)

    # shared (replicated) tensors
    vpad = cfg.VPAD
    E_pad = np.zeros((vpad, D), F32)
    E_pad[: cfg.V] = entity_emb
    et_tab = np.ascontiguousarray(E_pad.T).astype(BF16)            # [128, VPAD]
    e_sw = np.ascontiguousarray(
        E_pad.reshape(vpad // 512, 4, 128, D).transpose(0, 2, 1, 3)
    ).astype(BF16)                                                 # [VPAD/512,128,4,D]
    scale_pad = np.ones(vpad, F32)
    scale_pad[: cfg.V] = scale
    bias_pad = np.zeros(vpad, F32)
    bias_pad[: cfg.V] = bias
    scaleT = np.ascontiguousarray(scale_pad.reshape(cfg.NV, 128).T)
    biasT = np.ascontiguousarray(bias_pad.reshape(cfg.NV, 128).T)
    fast_relu = bool(np.all(scale == 1.0) and np.all(bias == 0.0))

    xb = x.astype(BF16)                                            # [N, D]
    rb = np.zeros((cfg.RPAD, D), F32)
    rb[: cfg.R] = rel_emb
    rb = rb.astype(BF16)

    shared = {
        "x_b": xb, "rel_b": rb, "et_tab": et_tab, "e_sw": e_sw,
        "scaleT": scaleT, "biasT": biasT,
    }
    in_maps = []
    for c in range(C):
        rows = slice(c * NPC, (c + 1) * NPC)
        at_c = np.ascontiguousarray(A[rows].T).astype(BF16)        # [N, NPC]
        rt_c = np.ascontiguousarray(Rm[rows].T).astype(BF16)       # [RPAD, NPC]
        m = dict(shared)
        m.update({"a_t": at_c, "r_t": rt_c})
        in_maps.append(m)
    return in_maps, fast_relu


def build(cfg, fast_relu, enable_asserts=False, dve_mod=2, dve_thresh=1):
    f32, bf16 = mybir.dt.float32, mybir.dt.bfloat16
    nc = bacc.Bacc(
        "TRN2", target_bir_lowering=False, debug=False,
        enable_asserts=enable_asserts,
    )
    D, NPC, NV = cfg.D, cfg.NPC, cfg.NV

    xb_t = nc.dram_tensor("x_b", [cfg.N, D], bf16, kind="ExternalInput").ap()
    rb_t = nc.dram_tensor("rel_b", [cfg.RPAD, D], bf16, kind="ExternalInput").ap()
    at_t = nc.dram_tensor("a_t", [cfg.N, NPC], bf16, kind="ExternalInput").ap()
    rt_t = nc.dram_tensor("r_t", [cfg.RPAD, NPC], bf16, kind="ExternalInput").ap()
    ett_t = nc.dram_tensor("et_tab", [128, cfg.VPAD], bf16, kind="ExternalInput").ap()
    esw_t = nc.dram_tensor("e_sw", [cfg.VPAD // 512, 128, 4, D], bf16, kind="ExternalInput").ap()
    scl_t = nc.dram_tensor("scaleT", [128, NV], f32, kind="ExternalInput").ap()
    bia_t = nc.dram_tensor("biasT", [128, NV], f32, kind="ExternalInput").ap()
    out_t = nc.dram_tensor("out", [128, NPC], f32, kind="ExternalOutput").ap()

    Relu = mybir.ActivationFunctionType.Relu

    with tile.TileContext(nc) as tc:
        with (
            tc.tile_pool(name="const", bufs=1) as constp,
            tc.tile_pool(name="aggk", bufs=4) as akp,
            tc.tile_pool(name="etab", bufs=6) as ep,
            tc.tile_pool(name="scoresb", bufs=6) as scp,
            tc.tile_pool(name="psA", bufs=1, space="PSUM") as psA,
            tc.tile_pool(name="psS", bufs=6, space="PSUM") as psS,
            tc.tile_pool(name="psO", bufs=1, space="PSUM") as psO,
        ):
            sclt = constp.tile([128, NV], f32, tag="sc")
            nc.sync.dma_start(sclt, scl_t)
            biat = constp.tile([128, NV], f32, tag="bi")
            nc.sync.dma_start(biat, bia_t)
            aggrT_sb = constp.tile([128, NPC], bf16, tag="aggrT")
            out_sb = constp.tile([128, NPC], f32, tag="outsb")

            # ---- phase 1: aggrT = x^T A^T + rel^T R^T  (k-chunked) --------
            aggr_ps = psA.tile([128, NPC], f32, tag="aggrps")
            for k in range(cfg.NKX):
                ks = slice(k * 128, (k + 1) * 128)
                xk = akp.tile([128, D], bf16, tag="lhs")
                nc.sync.dma_start(xk, xb_t[ks, :])
                ak = akp.tile([128, NPC], bf16, tag="rhs")
                nc.sync.dma_start(ak, at_t[ks, :])
                nc.tensor.matmul(
                    aggr_ps, lhsT=xk, rhs=ak,
                    start=(k == 0), stop=False, skip_group_check=True,
                )
            for k in range(cfg.NKR):
                ks = slice(k * 128, (k + 1) * 128)
                rk = akp.tile([128, D], bf16, tag="lhs")
                nc.sync.dma_start(rk, rb_t[ks, :])
                rrk = akp.tile([128, NPC], bf16, tag="rhs")
                nc.sync.dma_start(rrk, rt_t[ks, :])
                nc.tensor.matmul(
                    aggr_ps, lhsT=rk, rhs=rrk,
                    start=False, stop=(k == cfg.NKR - 1), skip_group_check=True,
                )
            nc.vector.tensor_copy(aggrT_sb, aggr_ps)

            # ---- phase 2: fused double matmul over entity chunks ----------
            outT_ps = psO.tile([128, NPC], f32, tag="outps")
            for vb in range(cfg.VPAD // 512):
                ett = ep.tile([128, 512], bf16, tag="et")
                nc.sync.dma_start(ett, ett_t[:, vb * 512 : (vb + 1) * 512])
                esw = ep.tile([128, 4, D], bf16, tag="ee")
                nc.sync.dma_start(esw, esw_t[vb])
                for j in range(4):
                    v = vb * 4 + j
                    sps = psS.tile([128, NPC], f32, tag="sps")
                    nc.tensor.matmul(
                        sps, lhsT=ett[:, j * 128 : (j + 1) * 128], rhs=aggrT_sb,
                        start=True, stop=True, skip_group_check=True,
                    )
                    st_sb = scp.tile([128, NPC], bf16, tag="st")
                    if fast_relu:
                        if v % dve_mod < dve_thresh:
                            nc.vector.tensor_relu(st_sb, sps)
                        else:
                            nc.scalar.activation(st_sb, sps, Relu)
                    else:
                        nc.scalar.activation(
                            st_sb, sps, Relu,
                            bias=biat[:, v : v + 1], scale=sclt[:, v : v + 1],
                        )
                    nc.tensor.matmul(
                        outT_ps, lhsT=esw[:, j, :], rhs=st_sb,
                        start=(v == 0), stop=(v == NV - 1), skip_group_check=True,
                    )

            nc.vector.tensor_copy(out_sb, outT_ps)
            nc.sync.dma_start(out_t, out_sb)

    nc.compile()
    return nc


def run(inputs, trace=False, cfg=None, dve_mod=2, dve_thresh=1):
    if cfg is None:
        cfg = Cfg()
    in_maps, fast_relu = host_prep(cfg, **inputs)
    nc = build(cfg, fast_relu, dve_mod=dve_mod, dve_thresh=dve_thresh)
    try:
        res = run_bass_kernel_spmd(
            nc, in_maps, core_ids=list(range(cfg.C)), trace=trace,
        )
    except ModuleNotFoundError:
        # NTFF profiling hook unavailable in this container; run untraced.
        res = run_bass_kernel_spmd(
            nc, in_maps, core_ids=list(range(cfg.C)), trace=False,
        )
    outs = []
    for c in range(cfg.C):
        outs.append(np.ascontiguousarray(np.asarray(res.results[c]["out"]).T))
    full = np.concatenate(outs, axis=0).astype(np.float32)
    return full, res


def kernel(**inputs):
    full, _ = run(inputs, trace=False)
    return full

